# revision 19
# baseline (speedup 1.0000x reference)
"""BitNet transformer block on 8 Trainium2 NeuronCores (Megatron tensor-parallel).

Self-contained: builds one SPMD Bass/Tile program, shards inputs on host,
runs via run_bass_kernel_spmd, gathers the output.

Sharding (per core c of 8):
  - wq/wk/wv column-parallel: output rows c*256:(c+1)*256 (= heads 2c, 2c+1)
  - wg/wu column-parallel: output rows c*1024:(c+1)*1024
  - wo/wd row-parallel: input cols c*256 / c*1024 slices
  - norms: row-parallel (each core normalizes its 256 rows, AllGathers the
    quantized activations in a pre-transposed block layout)

No DMA(xbar)-transposes are used anywhere: their completion semaphore was
observed (on HW) to fire before all SDMA engines landed their data when a
consumer is ready immediately, which corrupts prompt readers. Instead:
  - Quantized activations are PE-transposed locally (per 128x128 block)
    before the AllGather; the gathered [(core, kb, feat), row] layout is then
    read back with plain strided DMAs to form feature-major tiles.
  - Q and K are computed directly in head-dim-major layout by swapping the
    matmul operands (weights stationary, activations moving).
  - Attention-out and MLP-mid quantized tiles round-trip DRAM row-major and
    are PE-transposed on the way back in.

Numerics: quantized activations (ints in [-127,127]) and ternary weights are
exact in bf16; matmuls accumulate in fp32 PSUM, so every BitNet matmul is
exact integer arithmetic. Rounding uses the fp32 magic-constant trick
(+1.5*2**23) which matches jnp.round (ties-to-even). Attention runs in
S-transposed form (scores [k, q]); softmax skips the row-max subtraction
(scores for this model are O(1), exp cannot overflow) and the row sums come
from N=1 matmuls against a ones vector. Row-parallel matmul partials are
reduce-scattered in fp16 with the dequant scale folded into the PSUM
evacuation (validated offline: adds ~5e-4 relative noise).
"""

import os

import numpy as np
import ml_dtypes

import concourse.bacc as bacc
import concourse.mybir as mybir
import concourse.tile as tile
from concourse.bass_utils import run_bass_kernel_spmd

F32 = mybir.dt.float32
F16 = mybir.dt.float16
BF16 = mybir.dt.bfloat16
AF = mybir.ActivationFunctionType
ALU = mybir.AluOpType
AX = mybir.AxisListType

NCORES = 8
B, S, D, H, MLP = 2, int(os.environ.get("BITNET_S", "1024")), 2048, 16, \
    int(os.environ.get("BITNET_MLP", "8192"))
HD = 128
R = B * S                 # 2048 rows total
RL = R // NCORES          # 256 rows per core (row shard)
OQ = D // NCORES          # 256 qkv out cols per core (2 heads)
OM = MLP // NCORES        # 1024 mlp cols per core
P = 128
KT = D // P               # 16 feature chunks
RT = R // P               # 16 row tiles
LT = RL // P              # 2 local row tiles
ST = S // P               # 8 seq tiles per batch
NQC = ST // 4             # 512-col q chunks per (batch, head)
MAGIC = 12582912.0        # 1.5 * 2**23: fp32 round-to-nearest-even magic
INV_SQRT_HD = 1.0 / float(np.sqrt(HD))

_CACHED_NC = None


def _quant(nc, sp, src_ap, qscale_ap, out_bf_ap, wclip=False, tag="qtmp"):
    """out_bf = round(src * qscale) as bf16 (clip to [-1,1] if wclip)."""
    F = src_ap.shape[1]
    CH = min(F, 1024)
    for c0 in range(0, F, CH):
        tmp = sp.tile([src_ap.shape[0], CH], F32, tag=tag, name=tag)
        if qscale_ap is None:
            nc.vector.tensor_scalar(
                tmp[:], src_ap[:, c0:c0 + CH], MAGIC, None, op0=ALU.add
            )
        else:
            nc.vector.tensor_scalar(
                tmp[:], src_ap[:, c0:c0 + CH], qscale_ap, MAGIC,
                op0=ALU.mult, op1=ALU.add,
            )
        nc.scalar.activation(
            out_bf_ap[:, c0:c0 + CH], tmp[:], AF.Copy, bias=-MAGIC, scale=1.0
        )
    if wclip:
        nc.vector.tensor_scalar(
            out_bf_ap, out_bf_ap, 1.0, -1.0, op0=ALU.min, op1=ALU.max
        )


def _rms_quant_rows(nc, sp, ps_dummy, src_tile, nw_tile, as_out_ap, aq_out_ap):
    """rmsnorm + abs-max + int8-grid quantize for one [128, D] row tile."""
    sqd = ps_dummy.tile([P, D], F32, tag="sqd")
    ssq = sp.tile([P, 1], F32, tag="ssq")
    nc.scalar.activation(sqd[:], src_tile[:], AF.Square, accum_out=ssq[:])
    rms = sp.tile([P, 1], F32, tag="rms")
    nc.vector.tensor_scalar(
        rms[:], ssq[:], 1.0 / D, 1e-6, op0=ALU.mult, op1=ALU.add
    )
    nc.scalar.activation(rms[:], rms[:], AF.Sqrt)
    rinv = sp.tile([P, 1], F32, tag="rinv")
    nc.vector.reciprocal(rinv[:], rms[:])
    nc.vector.tensor_tensor(src_tile[:], src_tile[:], nw_tile[:], op=ALU.mult)
    amax = sp.tile([P, 1], F32, tag="amax")
    nc.vector.tensor_reduce(
        amax[:], src_tile[:], op=ALU.max, axis=AX.X, apply_absolute_value=True
    )
    nc.vector.tensor_scalar(
        as_out_ap, amax[:], rinv[:], 1e-8, op0=ALU.mult, op1=ALU.add
    )
    inva = sp.tile([P, 1], F32, tag="inva")
    nc.vector.reciprocal(inva[:], as_out_ap)
    qs = sp.tile([P, 1], F32, tag="qs")
    nc.vector.tensor_scalar(
        qs[:], inva[:], rinv[:], 127.0, op0=ALU.mult, op1=ALU.mult
    )
    _quant(nc, sp, src_tile[:], qs[:, 0:1], aq_out_ap)


def build_program():
    nc = bacc.Bacc(
        "TRN2",
        target_bir_lowering=False,
        debug=False,
        enable_asserts=True,
        num_devices=NCORES,
    )
    rg = [list(range(NCORES))]

    # ---------------- I/O ----------------
    x_rows = nc.dram_tensor("x_rows", [RL, D], F32, kind="ExternalInput").ap()
    wqkvT = nc.dram_tensor("wqkvT", [D, 3 * OQ], F32, kind="ExternalInput").ap()
    woT = nc.dram_tensor("woT", [OQ, D], F32, kind="ExternalInput").ap()
    wguT = nc.dram_tensor("wguT", [D, 2 * OM], F32, kind="ExternalInput").ap()
    wdT = nc.dram_tensor("wdT", [OM, D], F32, kind="ExternalInput").ap()
    norm1_w = nc.dram_tensor("norm1_w", [1, D], F32, kind="ExternalInput").ap()
    norm2_w = nc.dram_tensor("norm2_w", [1, D], F32, kind="ExternalInput").ap()
    mask01_b = nc.dram_tensor("mask01", [P, P], BF16, kind="ExternalInput").ap()
    ident_b = nc.dram_tensor("ident", [P, P], BF16, kind="ExternalInput").ap()
    wcnt_inv = nc.dram_tensor("wcnt_inv", [1, 8], F32, kind="ExternalInput").ap()
    out_d = nc.dram_tensor("out", [RL, D], F32, kind="ExternalOutput").ap()
    DBG = os.environ.get("BITNET_DEBUG") == "1"
    if DBG:
        dbg_qk = nc.dram_tensor("dbg_qk", [2 * P, R], BF16,
                                kind="ExternalOutput").ap()
        dbg_ao = nc.dram_tensor("dbg_ao", [R, OQ], BF16,
                                kind="ExternalOutput").ap()
        dbg_x1 = nc.dram_tensor("dbg_x1", [RL, D], F32,
                                kind="ExternalOutput").ap()
        dbg_m = nc.dram_tensor("dbg_m", [R, OM], BF16,
                               kind="ExternalOutput").ap()
        dbg_att = nc.dram_tensor("dbg_att", [R, 2 * P], F32,
                                 kind="ExternalOutput").ap()

    with tile.TileContext(nc) as tc, \
         tc.tile_pool(name="persist", bufs=1) as pp, \
         tc.tile_pool(name="dram", bufs=1, space="DRAM") as dp:

        # ---------------- constants ----------------
        mask01 = pp.tile([P, P], BF16, tag="mask01")
        nc.sync.dma_start(mask01[:], mask01_b)
        ident = pp.tile([P, P], BF16, tag="ident")
        nc.sync.dma_start(ident[:], ident_b)
        ones_bf = pp.tile([P, 1], BF16, tag="ones_bf")
        nc.vector.memset(ones_bf[:], 1.0)
        onesP = pp.tile([P, 1], F32, tag="onesP")
        nc.vector.memset(onesP[:], 1.0)
        wci = pp.tile([1, 8], F32, tag="wci")
        nc.sync.dma_start(wci[:], wcnt_inv)

        # persistent per-row-tile scale maps [P, RT]
        as1g = pp.tile([P, RT], F32, tag="as1g")
        cqT = pp.tile([P, RT], F32, tag="cqT")
        scv = pp.tile([P, RT], F32, tag="scv")
        aso = pp.tile([P, RT], F32, tag="aso")
        asog = pp.tile([P, RT], F32, tag="asog")
        qso = pp.tile([P, RT], F32, tag="qso")
        sc_oev = pp.tile([P, RT], F32, tag="sc_oev")
        as2g = pp.tile([P, RT], F32, tag="as2g")
        sc_g = pp.tile([P, RT], F32, tag="sc_g")
        sc_u = pp.tile([P, RT], F32, tag="sc_u")
        asm = pp.tile([P, RT], F32, tag="asm")
        asmg = pp.tile([P, RT], F32, tag="asmg")
        qsm = pp.tile([P, RT], F32, tag="qsm")
        sc_dev = pp.tile([P, RT], F32, tag="sc_dev")

        # ---------------- DRAM staging / collective buffers ----------------
        # agT layout: rows = (core, kb, feat) blocks, cols = local row index
        agT1_in = dp.tile([KT * P, RL], BF16, tag="agT1_in")
        agT1_out = dp.tile([NCORES * KT * P, RL], BF16, tag="agT1_out",
                           addr_space="Shared")
        ag1s_in = dp.tile([RL, 1], F32, tag="ag1s_in")
        ag1s_out = dp.tile([R, 1], F32, tag="ag1s_out", addr_space="Shared")
        wsa_in = dp.tile([1, 8], F32, tag="wsa_in")
        wsa_out = dp.tile([1, 8], F32, tag="wsa_out", addr_space="Shared")
        wsb_in = dp.tile([1, 8], F32, tag="wsb_in")
        wsb_out = dp.tile([1, 8], F32, tag="wsb_out", addr_space="Shared")
        aso_in = [dp.tile([R // 2, 1], F32, tag=f"aso_in{h}",
                          name=f"aso_in{h}") for h in range(2)]
        aso_g = [dp.tile([R // 2, 1], F32, tag=f"aso_g{h}", name=f"aso_g{h}",
                         addr_space="Shared") for h in range(2)]
        ao_d = dp.tile([R, OQ], BF16, tag="ao_d")
        x1_d = dp.tile([RL, D], F32, tag="x1_d")
        agT2_in = dp.tile([KT * P, RL], BF16, tag="agT2_in")
        agT2_out = dp.tile([NCORES * KT * P, RL], BF16, tag="agT2_out",
                           addr_space="Shared")
        ag2s_in = dp.tile([RL, 1], F32, tag="ag2s_in")
        ag2s_out = dp.tile([R, 1], F32, tag="ag2s_out", addr_space="Shared")
        m_d = dp.tile([R, OM], BF16, tag="m_d")

        NG = 4
        GT = RT // NG
        asm_in = [dp.tile([GT * P, 1], F32, tag=f"asmi{g}", name=f"asmi{g}")
                  for g in range(NG)]
        asm_go = [dp.tile([GT * P, 1], F32, tag=f"asmo{g}", name=f"asmo{g}",
                          addr_space="Shared")
                  for g in range(NG)]

        # weight-scale broadcast tiles (slot: 0=q 1=k 2=v 3=o 4=g 5=u 6=d)
        wsb = {}
        invb = {}
        for slot in range(7):
            wsb[slot] = pp.tile([P, 1], F32, tag=f"wsb{slot}", name=f"wsb{slot}")
            invb[slot] = pp.tile([P, 1], F32, tag=f"invb{slot}", name=f"invb{slot}")

        agT1v = agT1_out.rearrange("(c k f) r -> k f c r", c=NCORES, k=KT)
        agT2v = agT2_out.rearrange("(c k f) r -> k f c r", c=NCORES, k=KT)

        # =========================================================
        # poolB: QKV outputs + attention state (phase 1 .. attn-out quant)
        # =========================================================
        with tc.tile_pool(name="poolB", bufs=1) as pb:
            qkT = [pb.tile([P, R], BF16, tag=f"qkT{ot}", name=f"qkT{ot}")
                   for ot in range(4)]
            v_deq = {
                (b, hl, j): pb.tile([P, P], BF16, tag=f"vd{b}_{hl}_{j}",
                                    name=f"vd{b}_{hl}_{j}")
                for b in range(B) for hl in range(2) for j in range(ST)
            }
            attn_sb = [pb.tile([P, 2 * P], F32, tag=f"attn{t}", name=f"attn{t}")
                       for t in range(RT)]
            dkb = pb.tile([P, R], F32, tag="dkb")

            # -----------------------------------------------------
            # poolA: phase 1 + weight scales + wqkv quant + QKV MMs
            # -----------------------------------------------------
            with tc.tile_pool(name="poolA", bufs=1) as pa, \
                 tc.tile_pool(name="scrA", bufs=2) as s1:

                # ---- Phase 1: local rmsnorm1 + quant ----
                nw1 = pa.tile([P, D], F32, tag="nw1")
                nw1r = pa.tile([1, D], F32, tag="nw1r")
                nc.sync.dma_start(nw1r[:], norm1_w)
                nc.gpsimd.partition_broadcast(nw1[:], nw1r[0:1, :])
                aqs = [pa.tile([P, D], BF16, tag=f"aq{lt}", name=f"aq{lt}")
                       for lt in range(LT)]
                with tc.tile_pool(name="ps0", bufs=1, space="PSUM") as ps0:
                    for lt in range(LT):
                        xt = s1.tile([P, D], F32, tag="b2048f", name="b2048f")
                        nc.sync.dma_start(xt[:], x_rows[lt * P:(lt + 1) * P, :])
                        as_l = s1.tile([P, 1], F32, tag="as_l")
                        _rms_quant_rows(nc, s1, ps0, xt, nw1, as_l[:, 0:1],
                                        aqs[lt][:])
                        nc.sync.dma_start(
                            ag1s_in[lt * P:(lt + 1) * P, :], as_l[:]
                        )
                # PE-transpose local quant blocks into agT layout
                with tc.tile_pool(name="psT1", bufs=2, space="PSUM") as psT1:
                    for kb in range(KT):
                        atb = s1.tile([P, LT * P], BF16, tag="atb")
                        for lt in range(LT):
                            pst = psT1.tile([P, P], BF16, tag="pst")
                            nc.tensor.transpose(
                                pst[:], aqs[lt][:, kb * P:(kb + 1) * P],
                                ident[:],
                            )
                            if lt % 2 == 0:
                                nc.vector.tensor_copy(
                                    atb[:, lt * P:(lt + 1) * P], pst[:]
                                )
                            else:
                                nc.scalar.activation(
                                    atb[:, lt * P:(lt + 1) * P], pst[:],
                                    AF.Copy, bias=0.0, scale=1.0,
                                )
                        nc.sync.dma_start(
                            agT1_in[kb * P:(kb + 1) * P, :], atb[:]
                        )
                nc.gpsimd.collective_compute(
                    "AllGather", ALU.bypass, replica_groups=rg,
                    ins=[ag1s_in.opt()], outs=[ag1s_out.opt()],
                )

                # ---- weight |w| sums: pass 1 (streamed, gpsimd queue) ----
                W_SLOT_N = {0: KT, 1: KT, 2: KT, 3: OQ // P, 4: KT, 5: KT,
                            6: OM // P}
                accs = {
                    slot: pa.tile([P, n], F32, tag=f"acc{slot}",
                                  name=f"acc{slot}")
                    for slot, n in W_SLOT_N.items()
                }

                def finish_scales(out_dram, row_tag, slots):
                    grow = pa.tile([1, 8], F32, tag=row_tag, name=row_tag)
                    nc.sync.dma_start(grow[:], out_dram[:])
                    nc.vector.tensor_tensor(
                        grow[:], grow[:], wci[0:1, :], op=ALU.mult
                    )
                    nc.vector.tensor_scalar(
                        grow[:], grow[:], 1e-8, None, op0=ALU.add
                    )
                    girow = pa.tile([1, 8], F32, tag=row_tag + "i",
                                    name=row_tag + "i")
                    nc.vector.reciprocal(girow[:], grow[:])
                    for slot in slots:
                        nc.gpsimd.partition_broadcast(
                            wsb[slot][:], grow[0:1, slot:slot + 1]
                        )
                        nc.gpsimd.partition_broadcast(
                            invb[slot][:], girow[0:1, slot:slot + 1]
                        )

                def wsum_tensor(ap, nt, slots, bounds, tagp):
                    for i in range(nt):
                        wt = s1.tile([P, ap.shape[1]], F32, tag=tagp, name=tagp)
                        nc.gpsimd.dma_start(wt[:], ap[i * P:(i + 1) * P, :])
                        for slot, (lo, hi) in zip(slots, bounds):
                            nc.vector.tensor_reduce(
                                accs[slot][:, i:i + 1], wt[:, lo:hi],
                                op=ALU.add, axis=AX.X,
                                apply_absolute_value=True,
                            )

                wsum_tensor(wqkvT, KT, [0, 1, 2],
                            [(0, OQ), (OQ, 2 * OQ), (2 * OQ, 3 * OQ)], "w768")
                wsum_tensor(woT, OQ // P, [3], [(0, D)], "b2048f")
                svec = pa.tile([P, 8], F32, tag="svec")
                nc.vector.memset(svec[:], 0.0)
                for slot in range(4):
                    nc.vector.tensor_reduce(
                        svec[:, slot:slot + 1], accs[slot][:],
                        op=ALU.add, axis=AX.X,
                    )
                with tc.tile_pool(name="psw", bufs=1, space="PSUM") as psw:
                    tot_a = psw.tile([1, 8], F32, tag="tot_a")
                    nc.tensor.matmul(tot_a[:], onesP[:, 0:1], svec[:, 0:8],
                                     start=True, stop=True)
                    wrow_a = pa.tile([1, 8], F32, tag="wrow_a")
                    nc.vector.tensor_copy(wrow_a[:], tot_a[0:1, :])
                nc.sync.dma_start(wsa_in[:], wrow_a[:])
                nc.gpsimd.collective_compute(
                    "AllReduce", ALU.add, replica_groups=rg,
                    ins=[wsa_in.opt()], outs=[wsa_out.opt()],
                )
                nc.gpsimd.collective_compute(
                    "AllGather", ALU.bypass, replica_groups=rg,
                    ins=[agT1_in.opt()], outs=[agT1_out.opt()],
                )
                nc.sync.dma_start(
                    as1g[:], ag1s_out.rearrange("(t p) o -> p (t o)", p=P)
                )
                finish_scales(wsa_out, "gra", [0, 1, 2, 3])

                # ---- pass 2: quantize wqkv (re-streamed) ----
                wqkv_q = [
                    pa.tile([P, 3 * OQ], BF16, tag=f"wqkv_q{k}",
                            name=f"wqkv_q{k}")
                    for k in range(KT)
                ]
                for k in range(KT):
                    wt = s1.tile([P, 3 * OQ], F32, tag="w768", name="w768")
                    nc.gpsimd.dma_start(wt[:], wqkvT[k * P:(k + 1) * P, :])
                    for rr, slot in ((0, 0), (1, 1), (2, 2)):
                        _quant(
                            nc, s1, wt[:, rr * OQ:(rr + 1) * OQ],
                            invb[slot][:, 0:1],
                            wqkv_q[k][:, rr * OQ:(rr + 1) * OQ],
                            wclip=True, tag="qtmp",
                        )
                # a1T[kb]: plain strided loads from agT1_out
                a1T = [pa.tile([P, R], BF16, tag=f"a1T{kb}", name=f"a1T{kb}")
                       for kb in range(KT)]
                for kb in range(KT):
                    eng = nc.sync if kb % 2 == 0 else nc.scalar
                    eng.dma_start(a1T[kb][:], agT1v[kb])

                wsum_tensor(wguT, KT, [4, 5], [(0, OM), (OM, 2 * OM)],
                            "b2048f")
                wsum_tensor(wdT, OM // P, [6], [(0, D)], "b2048f")
                svecb = pa.tile([P, 8], F32, tag="svecb")
                nc.vector.memset(svecb[:], 0.0)
                for slot in range(4, 7):
                    nc.vector.tensor_reduce(
                        svecb[:, slot:slot + 1], accs[slot][:],
                        op=ALU.add, axis=AX.X,
                    )
                with tc.tile_pool(name="pswb", bufs=1, space="PSUM") as pswb:
                    tot_b = pswb.tile([1, 8], F32, tag="tot_b")
                    nc.tensor.matmul(tot_b[:], onesP[:, 0:1], svecb[:, 0:8],
                                     start=True, stop=True)
                    wrow_b = pa.tile([1, 8], F32, tag="wrow_b")
                    nc.vector.tensor_copy(wrow_b[:], tot_b[0:1, :])
                nc.sync.dma_start(wsb_in[:], wrow_b[:])
                nc.gpsimd.collective_compute(
                    "AllReduce", ALU.add, replica_groups=rg,
                    ins=[wsb_in.opt()], outs=[wsb_out.opt()],
                )

                finish_scales(wsb_out, "grb", [4, 5, 6])

                # attention dequant scale maps
                pw = pa.tile([P, 1], F32, tag="pw")
                nc.vector.tensor_tensor(pw[:], wsb[0][:], wsb[1][:], op=ALU.mult)
                nc.vector.tensor_scalar(
                    pw[:], pw[:], INV_SQRT_HD / (127.0 * 127.0), None,
                    op0=ALU.mult,
                )
                nc.vector.tensor_scalar(
                    cqT[:], as1g[:], pw[:, 0:1], None, op0=ALU.mult
                )
                nc.vector.tensor_scalar(
                    scv[:], as1g[:], wsb[2][:, 0:1], 1.0 / 127.0,
                    op0=ALU.mult, op1=ALU.mult,
                )
                dk_row = pa.tile([1, R], F32, tag="dk_row")
                nc.sync.dma_start(dk_row[:], ag1s_out.rearrange("r o -> o r"))
                nc.gpsimd.partition_broadcast(dkb[:], dk_row[0:1, :])

                # ---- QKV matmuls ----
                # Q/K: transposed outputs via operand swap (weights stationary)
                with tc.tile_pool(name="ps2", bufs=1, space="PSUM") as ps2, \
                     tc.tile_pool(name="ps2v", bufs=3, space="PSUM") as ps2v:
                    NRC = R // 512
                    for ot in range(4):
                        psqk4 = [ps2.tile([P, 512], F32, tag=f"psqk{rc}",
                                          name=f"psqk{rc}")
                                 for rc in range(NRC)]
                        for kb in range(KT):
                            for rc in range(NRC):
                                nc.tensor.matmul(
                                    psqk4[rc][:],
                                    wqkv_q[kb][:, ot * P:(ot + 1) * P],
                                    a1T[kb][:, rc * 512:(rc + 1) * 512],
                                    start=(kb == 0), stop=(kb == KT - 1),
                                )
                        for rc in range(NRC):
                            if (ot + rc) % 2 == 0:
                                nc.vector.tensor_copy(
                                    qkT[ot][:, rc * 512:(rc + 1) * 512],
                                    psqk4[rc][:],
                                )
                            else:
                                nc.scalar.activation(
                                    qkT[ot][:, rc * 512:(rc + 1) * 512],
                                    psqk4[rc][:], AF.Copy, bias=0.0, scale=1.0,
                                )
                    # V: row-major (needed as [krow, hd] for att@V)
                    for t in range(RT):
                        psv = ps2v.tile([P, 2 * P], F32, tag="psv")
                        for kb in range(KT):
                            nc.tensor.matmul(
                                psv[:], a1T[kb][:, t * P:(t + 1) * P],
                                wqkv_q[kb][:, 512:768],
                                start=(kb == 0), stop=(kb == KT - 1),
                            )
                        b, j = divmod(t, ST)
                        for hl in range(2):
                            nc.vector.tensor_scalar(
                                v_deq[(b, hl, j)][:],
                                psv[:, hl * P:(hl + 1) * P],
                                scv[:, t:t + 1], None, op0=ALU.mult,
                            )
            # poolA (wqkv_q, a1T, weight-sum accs) frees here

            if DBG:
                nc.sync.dma_start(dbg_qk[0:P, :], qkT[0][:])
                nc.sync.dma_start(dbg_qk[P:2 * P, :], qkT[2][:])

            # ---- attention (S^T form) ----
            with tc.tile_pool(name="scrB", bufs=2) as s2, \
                 tc.tile_pool(name="psA", bufs=1, space="PSUM") as psA, \
                 tc.tile_pool(name="psS", bufs=2, space="PSUM") as psS:
                att4 = [psA.tile([P, 512], F32, tag=f"att{ii}",
                                 name=f"att{ii}") for ii in range(4)]
                esum_ps = psA.tile([P, 512], F32, tag="esum_ps")
                for b in range(B):
                    for hl in range(2):
                        for hf in range(NQC):
                            q0 = 4 * hf
                            jmax = q0 + 3
                            qc0 = b * S + hf * 512
                            for j in range(jmax + 1):
                                ps_s = psS.tile([P, 512], F32, tag="ps_s")
                                nc.tensor.matmul(
                                    ps_s[:],
                                    qkT[2 + hl][:, b * S + j * P:
                                                b * S + (j + 1) * P],
                                    qkT[hl][:, qc0:qc0 + 512],
                                    start=True, stop=True,
                                )
                                s1t = s2.tile([P, 512], F32, tag="s1t")
                                nc.vector.scalar_tensor_tensor(
                                    s1t[:], ps_s[:],
                                    cqT[:, b * ST + j:b * ST + j + 1],
                                    dkb[:, qc0:qc0 + 512],
                                    op0=ALU.mult, op1=ALU.mult,
                                )
                                pt = s2.tile([P, 512], BF16, tag="pt")
                                nc.scalar.activation(
                                    pt[:], s1t[:], AF.Exp, bias=0.0, scale=1.0
                                )
                                if j >= q0:
                                    dc = (j - q0) * P
                                    nc.vector.tensor_tensor(
                                        pt[:, dc:dc + P], pt[:, dc:dc + P],
                                        mask01[:], op=ALU.mult,
                                    )
                                for ii in range(4):
                                    i = q0 + ii
                                    if j > i:
                                        continue
                                    lhsT = pt[:, ii * P:(ii + 1) * P]
                                    nc.tensor.matmul(
                                        att4[ii][:, 0:P], lhsT,
                                        v_deq[(b, hl, j)][:],
                                        start=(j == 0), stop=(j == i),
                                    )
                                    ec = ii * (jmax + 1) + j
                                    nc.tensor.matmul(
                                        esum_ps[:, ec:ec + 1], lhsT,
                                        ones_bf[:, 0:1],
                                        start=True, stop=True,
                                    )
                            for ii in range(4):
                                i = q0 + ii
                                t = b * ST + i
                                e0 = ii * (jmax + 1)
                                es = s2.tile([P, 1], F32, tag="es")
                                nc.vector.tensor_reduce(
                                    es[:], esum_ps[:, e0:e0 + i + 1],
                                    op=ALU.add, axis=AX.X,
                                )
                                rec = s2.tile([P, 1], F32, tag="rec")
                                nc.vector.reciprocal(rec[:], es[:])
                                nc.vector.tensor_scalar(
                                    attn_sb[t][:, hl * P:(hl + 1) * P],
                                    att4[ii][:, 0:P], rec[:, 0:1], None,
                                    op0=ALU.mult,
                                )

                if DBG:
                    for t in range(RT):
                        nc.sync.dma_start(
                            dbg_att[t * P:(t + 1) * P, :], attn_sb[t][:]
                        )
                # ---- a_scale_o: AllReduce-max over heads, per row-half
                # (half 0 overlaps batch-1 attention) ----
                HT = RT // 2
                for h in range(2):
                    t0, t1 = h * HT, (h + 1) * HT
                    for t in range(t0, t1):
                        nc.vector.tensor_reduce(
                            aso[:, t:t + 1], attn_sb[t][:], op=ALU.max,
                            axis=AX.X, apply_absolute_value=True,
                        )
                    nc.sync.dma_start(
                        aso_in[h].rearrange("(t p) o -> p (t o)", p=P),
                        aso[:, t0:t1],
                    )
                    nc.gpsimd.collective_compute(
                        "AllReduce", ALU.max, replica_groups=rg,
                        ins=[aso_in[h].opt()], outs=[aso_g[h].opt()],
                    )
                    nc.sync.dma_start(
                        asog[:, t0:t1],
                        aso_g[h].rearrange("(t p) o -> p (t o)", p=P),
                    )
                    nc.vector.tensor_scalar(
                        asog[:, t0:t1], asog[:, t0:t1], 1e-8, None, op0=ALU.add
                    )
                    nc.vector.reciprocal(qso[:, t0:t1], asog[:, t0:t1])
                    nc.vector.tensor_scalar(
                        qso[:, t0:t1], qso[:, t0:t1], 127.0, None, op0=ALU.mult
                    )
                    nc.vector.tensor_scalar(
                        sc_oev[:, t0:t1], asog[:, t0:t1], wsb[3][:, 0:1],
                        1.0 / 127.0, op0=ALU.mult, op1=ALU.mult,
                    )
                    for t in range(t0, t1):
                        a_qo = s2.tile([P, 2 * P], BF16, tag="a_qo")
                        _quant(nc, s2, attn_sb[t][:], qso[:, t:t + 1], a_qo[:],
                               tag="qotmp")
                        nc.sync.dma_start(
                            ao_d[t * P:(t + 1) * P, :], a_qo[:]
                        )
        # poolB frees here

        # =========================================================
        # poolC: o-proj + MLP weights (wgu/wd quant overlaps o-proj)
        # =========================================================
        rs1_out = []
        with tc.tile_pool(name="poolC", bufs=1) as pc:
            wgu_q = [pc.tile([P, 2 * OM], BF16, tag=f"wgu_q{k}",
                             name=f"wgu_q{k}") for k in range(KT)]
            wd_q = [pc.tile([P, D], BF16, tag=f"wd_q{kb}", name=f"wd_q{kb}")
                    for kb in range(OM // P)]

            with tc.tile_pool(name="poolC2", bufs=1) as pc2, \
                 tc.tile_pool(name="scrC", bufs=2) as s3, \
                 tc.tile_pool(name="ps5", bufs=3, space="PSUM") as ps5, \
                 tc.tile_pool(name="psT2", bufs=2, space="PSUM") as psT2:
                # wo quant (re-streamed)
                wo_q = [pc2.tile([P, D], BF16, tag=f"wo_q{kk}",
                                 name=f"wo_q{kk}") for kk in range(OQ // P)]
                for kk in range(OQ // P):
                    wt = s3.tile([P, D], F32, tag="c2048f", name="c2048f")
                    nc.gpsimd.dma_start(wt[:], woT[kk * P:(kk + 1) * P, :])
                    _quant(nc, s3, wt[:], invb[3][:, 0:1], wo_q[kk][:],
                           wclip=True, tag="qtmpC")
                # a_oT via PE transposes of re-loaded quantized tiles
                a_oT = [pc2.tile([P, R], BF16, tag=f"a_oT{kk}",
                                 name=f"a_oT{kk}") for kk in range(OQ // P)]
                for t in range(RT):
                    aor = s3.tile([P, OQ], BF16, tag="aor")
                    nc.sync.dma_start(aor[:], ao_d[t * P:(t + 1) * P, :])
                    for kk in range(OQ // P):
                        pst = psT2.tile([P, P], BF16, tag="pst2")
                        nc.tensor.transpose(
                            pst[:], aor[:, kk * P:(kk + 1) * P], ident[:]
                        )
                        if kk % 2 == 0:
                            nc.vector.tensor_copy(
                                a_oT[kk][:, t * P:(t + 1) * P], pst[:]
                            )
                        else:
                            nc.scalar.activation(
                                a_oT[kk][:, t * P:(t + 1) * P], pst[:],
                                AF.Copy, bias=0.0, scale=1.0,
                            )

                # ---- o-proj: int matmul, dequant folded in evac, fp16 RS ----
                for oc in range(2):
                    rs_in = dp.tile([R, D // 2], F16, tag=f"rs1i{oc}")
                    rs_out = dp.tile([RL, D // 2], F16, tag=f"rs1o{oc}")
                    rs1_out.append(rs_out)
                    for t in range(RT):
                        pso = ps5.tile([P, D // 2], F32, tag="ops")
                        for kk in range(OQ // P):
                            lhsT = a_oT[kk][:, t * P:(t + 1) * P]
                            for n in range(2):
                                c0 = oc * (D // 2) + n * 512
                                nc.tensor.matmul(
                                    pso[:, n * 512:(n + 1) * 512], lhsT,
                                    wo_q[kk][:, c0:c0 + 512],
                                    start=(kk == 0), stop=(kk == OQ // P - 1),
                                )
                        osb = s3.tile([P, D // 2], F16, tag="osb")
                        nc.scalar.activation(
                            osb[:], pso[:], AF.Copy, bias=0.0,
                            scale=sc_oev[:, t:t + 1],
                        )
                        nc.sync.dma_start(rs_in[t * P:(t + 1) * P, :], osb[:])
                    nc.gpsimd.collective_compute(
                        "ReduceScatter", ALU.add, replica_groups=rg,
                        ins=[rs_in.opt()], outs=[rs_out.opt()],
                    )

                # ---- wgu/wd pass 2 + quant (fills o-proj/phase-4 gaps) ----
                for k in range(KT):
                    wt = s3.tile([P, 2 * OM], F32, tag="c2048f", name="c2048f")
                    nc.gpsimd.dma_start(wt[:], wguT[k * P:(k + 1) * P, :])
                    _quant(nc, s3, wt[:, 0:OM], invb[4][:, 0:1],
                           wgu_q[k][:, 0:OM], wclip=True, tag="qtmpC")
                    _quant(nc, s3, wt[:, OM:2 * OM], invb[5][:, 0:1],
                           wgu_q[k][:, OM:2 * OM], wclip=True, tag="qtmpC")
                for kb in range(OM // P):
                    wt = s3.tile([P, D], F32, tag="c2048f", name="c2048f")
                    nc.gpsimd.dma_start(wt[:], wdT[kb * P:(kb + 1) * P, :])
                    _quant(nc, s3, wt[:], invb[6][:, 0:1], wd_q[kb][:],
                           wclip=True, tag="qtmpC")
            # poolC2 (wo_q, a_oT) frees here

            # =========================================================
            # Phase 4: residual + rmsnorm2 + quant + AG2 (agT layout)
            # =========================================================
            with tc.tile_pool(name="p4", bufs=1) as p4, \
                 tc.tile_pool(name="s4", bufs=2) as s4, \
                 tc.tile_pool(name="ps40", bufs=1, space="PSUM") as ps40, \
                 tc.tile_pool(name="psT4", bufs=2, space="PSUM") as psT4:
                nw2 = p4.tile([P, D], F32, tag="nw2")
                nw2r = p4.tile([1, D], F32, tag="nw2r")
                nc.sync.dma_start(nw2r[:], norm2_w)
                nc.gpsimd.partition_broadcast(nw2[:], nw2r[0:1, :])
                aq2s = [p4.tile([P, D], BF16, tag=f"aq2{lt}", name=f"aq2{lt}")
                        for lt in range(LT)]
                for lt in range(LT):
                    x1t = s4.tile([P, D], F32, tag="x1t", bufs=1)
                    xr = p4.tile([P, D], F32, tag="xr")
                    nc.sync.dma_start(xr[:], x_rows[lt * P:(lt + 1) * P, :])
                    for oc in range(2):
                        ysb = s4.tile([P, D // 2], F16, tag="ysb")
                        nc.sync.dma_start(
                            ysb[:], rs1_out[oc][lt * P:(lt + 1) * P, :]
                        )
                        nc.vector.tensor_tensor(
                            x1t[:, oc * (D // 2):(oc + 1) * (D // 2)],
                            ysb[:], xr[:, oc * (D // 2):(oc + 1) * (D // 2)],
                            op=ALU.add,
                        )
                    nc.sync.dma_start(x1_d[lt * P:(lt + 1) * P, :], x1t[:])
                    as_l = p4.tile([P, 1], F32, tag="as_l2")
                    _rms_quant_rows(nc, s4, ps40, x1t, nw2, as_l[:, 0:1],
                                    aq2s[lt][:])
                    nc.sync.dma_start(
                        ag2s_in[lt * P:(lt + 1) * P, :], as_l[:]
                    )
                for kb in range(KT):
                    atb = s4.tile([P, LT * P], BF16, tag="atb2")
                    for lt in range(LT):
                        pst = psT4.tile([P, P], BF16, tag="pst4")
                        nc.tensor.transpose(
                            pst[:], aq2s[lt][:, kb * P:(kb + 1) * P], ident[:]
                        )
                        if lt % 2 == 0:
                            nc.vector.tensor_copy(
                                atb[:, lt * P:(lt + 1) * P], pst[:]
                            )
                        else:
                            nc.scalar.activation(
                                atb[:, lt * P:(lt + 1) * P], pst[:],
                                AF.Copy, bias=0.0, scale=1.0,
                            )
                    nc.sync.dma_start(
                        agT2_in[kb * P:(kb + 1) * P, :], atb[:]
                    )

            nc.gpsimd.collective_compute(
                "AllGather", ALU.bypass, replica_groups=rg,
                ins=[ag2s_in.opt()], outs=[ag2s_out.opt()],
            )
            nc.gpsimd.collective_compute(
                "AllGather", ALU.bypass, replica_groups=rg,
                ins=[agT2_in.opt()], outs=[agT2_out.opt()],
            )

            # =========================================================
            # MLP: gate/up, m quant, down, fp16 RS
            # =========================================================
            nc.sync.dma_start(
                as2g[:], ag2s_out.rearrange("(t p) o -> p (t o)", p=P)
            )
            nc.vector.tensor_scalar(
                sc_g[:], as2g[:], wsb[4][:, 0:1], 1.0 / 127.0,
                op0=ALU.mult, op1=ALU.mult,
            )
            nc.vector.tensor_scalar(
                sc_u[:], as2g[:], wsb[5][:, 0:1], 1.0 / 127.0,
                op0=ALU.mult, op1=ALU.mult,
            )

            with tc.tile_pool(name="poolE", bufs=1) as pe, \
                 tc.tile_pool(name="scrE", bufs=2) as s5:
                a2T = [pe.tile([P, R], BF16, tag=f"a2T{kb}", name=f"a2T{kb}")
                       for kb in range(KT)]
                for kb in range(KT):
                    eng = nc.sync if kb % 2 == 0 else nc.scalar
                    eng.dma_start(a2T[kb][:], agT2v[kb])
                m_tiles = [pe.tile([P, OM], F32, tag=f"m{t % 5}",
                                   name=f"m{t % 5}") for t in range(RT)]
                with tc.tile_pool(name="ps6", bufs=2, space="PSUM") as ps6:
                    for g in range(NG):
                        for tl in range(GT):
                            t = g * GT + tl
                            psg = ps6.tile([P, 2 * OM], F32, tag="psg")
                            for kb in range(KT):
                                lhsT = a2T[kb][:, t * P:(t + 1) * P]
                                for n in range(2 * OM // 512):
                                    nc.tensor.matmul(
                                        psg[:, n * 512:(n + 1) * 512], lhsT,
                                        wgu_q[kb][:, n * 512:(n + 1) * 512],
                                        start=(kb == 0), stop=(kb == KT - 1),
                                    )
                            sig = s5.tile([P, OM], F32, tag="sig")
                            nc.scalar.activation(
                                sig[:], psg[:, 0:OM], AF.Sigmoid,
                                scale=sc_g[:, t:t + 1],
                            )
                            sgl = s5.tile([P, OM], F32, tag="sgl", bufs=1)
                            nc.vector.scalar_tensor_tensor(
                                sgl[:], psg[:, 0:OM], sc_g[:, t:t + 1], sig[:],
                                op0=ALU.mult, op1=ALU.mult,
                            )
                            nc.vector.scalar_tensor_tensor(
                                m_tiles[t][:], psg[:, OM:2 * OM],
                                sc_u[:, t:t + 1], sgl[:],
                                op0=ALU.mult, op1=ALU.mult,
                            )
                            nc.vector.tensor_reduce(
                                asm[:, t:t + 1], m_tiles[t][:], op=ALU.max,
                                axis=AX.X, apply_absolute_value=True,
                            )
                        nc.sync.dma_start(
                            asm_in[g].rearrange("(t p) o -> p (t o)", p=P),
                            asm[:, g * GT:(g + 1) * GT],
                        )
                        nc.gpsimd.collective_compute(
                            "AllReduce", ALU.max, replica_groups=rg,
                            ins=[asm_in[g].opt()], outs=[asm_go[g].opt()],
                        )
                        nc.sync.dma_start(
                            asmg[:, g * GT:(g + 1) * GT],
                            asm_go[g].rearrange("(t p) o -> p (t o)", p=P),
                        )
                        nc.vector.tensor_scalar(
                            asmg[:, g * GT:(g + 1) * GT],
                            asmg[:, g * GT:(g + 1) * GT], 1e-8, None,
                            op0=ALU.add,
                        )
                        nc.vector.reciprocal(
                            qsm[:, g * GT:(g + 1) * GT],
                            asmg[:, g * GT:(g + 1) * GT],
                        )
                        nc.vector.tensor_scalar(
                            qsm[:, g * GT:(g + 1) * GT],
                            qsm[:, g * GT:(g + 1) * GT], 127.0, None,
                            op0=ALU.mult,
                        )
                        for tl in range(GT):
                            t = g * GT + tl
                            m_q = s5.tile([P, OM], BF16, tag="m_q", bufs=1)
                            _quant(nc, s5, m_tiles[t][:], qsm[:, t:t + 1],
                                   m_q[:], tag="qtmpE")
                            nc.sync.dma_start(
                                m_d[t * P:(t + 1) * P, :], m_q[:]
                            )
            # poolE (a2T, m_tiles) frees here

            nc.vector.tensor_scalar(
                sc_dev[:], asmg[:], wsb[6][:, 0:1], 1.0 / 127.0,
                op0=ALU.mult, op1=ALU.mult,
            )
            with tc.tile_pool(name="poolF", bufs=1) as pf, \
                 tc.tile_pool(name="scrF", bufs=2) as s6, \
                 tc.tile_pool(name="psT3", bufs=2, space="PSUM") as psT3:
                # mT via PE transposes of re-loaded quantized m tiles
                mT = [pf.tile([P, R], BF16, tag=f"mT{kb}", name=f"mT{kb}")
                      for kb in range(OM // P)]
                for t in range(RT):
                    mrow = s6.tile([P, OM], BF16, tag="mrow")
                    eng = nc.sync if t % 2 == 0 else nc.scalar
                    eng.dma_start(mrow[:], m_d[t * P:(t + 1) * P, :])
                    for kb in range(OM // P):
                        pst = psT3.tile([P, P], BF16, tag="pst3")
                        nc.tensor.transpose(
                            pst[:], mrow[:, kb * P:(kb + 1) * P], ident[:]
                        )
                        if kb % 2 == 0:
                            nc.vector.tensor_copy(
                                mT[kb][:, t * P:(t + 1) * P], pst[:]
                            )
                        else:
                            nc.scalar.activation(
                                mT[kb][:, t * P:(t + 1) * P], pst[:],
                                AF.Copy, bias=0.0, scale=1.0,
                            )

                rs2_out = []
                with tc.tile_pool(name="ps7", bufs=3, space="PSUM") as ps7:
                    for oc in range(2):
                        rs_in = dp.tile([R, D // 2], F16, tag=f"rs2i{oc}")
                        rs_out = dp.tile([RL, D // 2], F16, tag=f"rs2o{oc}")
                        rs2_out.append(rs_out)
                        for t in range(RT):
                            psd = ps7.tile([P, D // 2], F32, tag="dps")
                            for kb in range(OM // P):
                                lhsT = mT[kb][:, t * P:(t + 1) * P]
                                for n in range(2):
                                    c0 = oc * (D // 2) + n * 512
                                    nc.tensor.matmul(
                                        psd[:, n * 512:(n + 1) * 512], lhsT,
                                        wd_q[kb][:, c0:c0 + 512],
                                        start=(kb == 0),
                                        stop=(kb == OM // P - 1),
                                    )
                            dsb = s6.tile([P, D // 2], F16, tag="dsb")
                            nc.scalar.activation(
                                dsb[:], psd[:], AF.Copy, bias=0.0,
                                scale=sc_dev[:, t:t + 1],
                            )
                            nc.sync.dma_start(
                                rs_in[t * P:(t + 1) * P, :], dsb[:]
                            )
                        nc.gpsimd.collective_compute(
                            "ReduceScatter", ALU.add, replica_groups=rg,
                            ins=[rs_in.opt()], outs=[rs_out.opt()],
                        )

                for lt in range(LT):
                    x1r = s6.tile([P, D], F32, tag="x1r", bufs=1)
                    nc.sync.dma_start(x1r[:], x1_d[lt * P:(lt + 1) * P, :])
                    ot = s6.tile([P, D], F32, tag="otile", bufs=1)
                    for oc in range(2):
                        ysb = s6.tile([P, D // 2], F16, tag="ysb2")
                        nc.sync.dma_start(
                            ysb[:], rs2_out[oc][lt * P:(lt + 1) * P, :]
                        )
                        nc.vector.tensor_tensor(
                            ot[:, oc * (D // 2):(oc + 1) * (D // 2)], ysb[:],
                            x1r[:, oc * (D // 2):(oc + 1) * (D // 2)],
                            op=ALU.add,
                        )
                    nc.sync.dma_start(out_d[lt * P:(lt + 1) * P, :], ot[:])

            if DBG:
                nc.sync.dma_start(dbg_ao[:, :], ao_d[:, :])
                nc.sync.dma_start(dbg_x1[:, :], x1_d[:, :])
                nc.sync.dma_start(dbg_m[:, :], m_d[:, :])

    nc.compile()
    return nc


def _prep_in_maps(inputs):
    x = np.asarray(inputs["x"], np.float32).reshape(R, D)
    wq = np.asarray(inputs["wq"], np.float32)
    wk = np.asarray(inputs["wk"], np.float32)
    wv = np.asarray(inputs["wv"], np.float32)
    wo = np.asarray(inputs["wo"], np.float32)
    wg = np.asarray(inputs["wg"], np.float32)
    wu = np.asarray(inputs["wu"], np.float32)
    wd = np.asarray(inputs["wd"], np.float32)
    n1 = np.asarray(inputs["norm1_w"], np.float32).reshape(1, D)
    n2 = np.asarray(inputs["norm2_w"], np.float32).reshape(1, D)

    kv, qv = np.mgrid[0:P, 0:P]
    mask01 = (kv <= qv).astype(ml_dtypes.bfloat16)   # S^T: valid where k <= q
    ident = np.eye(P, dtype=ml_dtypes.bfloat16)
    wcnt = np.array(
        [[D * D, D * D, D * D, D * D, MLP * D, MLP * D, D * MLP, 1.0]],
        np.float64,
    )
    wcnt_inv = (1.0 / wcnt).astype(np.float32)

    in_maps = []
    for c in range(NCORES):
        qs = slice(c * OQ, (c + 1) * OQ)
        ms = slice(c * OM, (c + 1) * OM)
        in_maps.append({
            "x_rows": np.ascontiguousarray(x[c * RL:(c + 1) * RL]),
            "wqkvT": np.ascontiguousarray(
                np.concatenate([wq[qs], wk[qs], wv[qs]], 0).T
            ),
            "woT": np.ascontiguousarray(wo[:, qs].T),
            "wguT": np.ascontiguousarray(
                np.concatenate([wg[ms], wu[ms]], 0).T
            ),
            "wdT": np.ascontiguousarray(wd[:, ms].T),
            "norm1_w": n1,
            "norm2_w": n2,
            "mask01": mask01,
            "ident": ident,
            "wcnt_inv": wcnt_inv,
        })
    return in_maps


def kernel(**inputs) -> np.ndarray:
    global _CACHED_NC
    if _CACHED_NC is None:
        _CACHED_NC = build_program()
    nc = _CACHED_NC
    in_maps = _prep_in_maps(inputs)
    res = run_bass_kernel_spmd(nc, in_maps, core_ids=list(range(NCORES)))
    out = np.concatenate([res.results[c]["out"] for c in range(NCORES)], 0)
    return out.reshape(B, S, D).astype(np.float32)
